# revision 1
# baseline (speedup 1.0000x reference)
"""Trainium2 Bass kernel for ComplexProjection:
    out[b,r,p] = |sum_s complex(x_real,x_imag)[b,r,s] * projection[r,s,p]|

Strategy: data-parallel over the particle axis B across 8 NeuronCores.
Each core computes, for its B-shard (Bc=4096) and every r:
    re[p,b] = sum_s w[r,s,p] * x_real[b,r,s]   (PE matmul, W stationary)
    im[p,b] = sum_s w[r,s,p] * x_imag[b,r,s]
    out[p,b] = sqrt(re^2 + im^2)               (ACT square/sqrt + DVE add)
The contraction dim S must live on SBUF partitions for both matmul
operands, so the host passes the x shards pre-transposed to [r, s, b]
(cheap numpy work; device time is what counts) and receives the output
as [r, p, b], which the host permutes back.
"""

import numpy as np

B, R, S, P = 32768, 16, 128, 128
NCORES = 8
BC = B // NCORES  # 4096 particles per core
CH = 512          # matmul moving-dim chunk (one PSUM bank of fp32)
NCH = BC // CH

_prog_cache = {}


def _build_program():
    if "nc" in _prog_cache:
        return _prog_cache["nc"]

    import concourse.tile as tile
    from concourse import bacc, mybir

    nc = bacc.Bacc("TRN2", target_bir_lowering=False, debug=False,
                   num_devices=NCORES)
    xr = nc.dram_tensor("xr", [R, S, BC], mybir.dt.float32, kind="ExternalInput")
    xi = nc.dram_tensor("xi", [R, S, BC], mybir.dt.float32, kind="ExternalInput")
    w = nc.dram_tensor("w", [R, S, P], mybir.dt.float32, kind="ExternalInput")
    o = nc.dram_tensor("o", [R, P, BC], mybir.dt.float32, kind="ExternalOutput")
    xr_ap, xi_ap, w_ap, o_ap = xr.ap(), xi.ap(), w.ap(), o.ap()

    f32 = mybir.dt.float32
    with tile.TileContext(nc) as tc:
        with (
            tc.tile_pool(name="wp", bufs=1) as wp,
            tc.tile_pool(name="xp", bufs=2) as xp,
            tc.tile_pool(name="op", bufs=2) as op,
            tc.tile_pool(name="sq", bufs=3) as sqp,
            tc.tile_pool(name="ps", bufs=2, space="PSUM") as psp,
        ):
            # All R projection matrices stay resident: [s=128, r, p]
            w_sb = wp.tile([S, R, P], f32)
            for r in range(R):
                nc.sync.dma_start(w_sb[:, r, :], w_ap[r])

            for r in range(R):
                xr_sb = xp.tile([S, BC], f32, tag="xr")
                nc.sync.dma_start(xr_sb[:], xr_ap[r])
                xi_sb = xp.tile([S, BC], f32, tag="xi")
                nc.sync.dma_start(xi_sb[:], xi_ap[r])
                out_sb = op.tile([P, BC], f32)
                for c in range(NCH):
                    sl = slice(c * CH, (c + 1) * CH)
                    ps_r = psp.tile([P, CH], f32, tag="psr")
                    nc.tensor.matmul(ps_r[:], w_sb[:, r, :], xr_sb[:, sl],
                                     start=True, stop=True)
                    ps_i = psp.tile([P, CH], f32, tag="psi")
                    nc.tensor.matmul(ps_i[:], w_sb[:, r, :], xi_sb[:, sl],
                                     start=True, stop=True)
                    sq_r = sqp.tile([P, CH], f32, tag="sqr")
                    nc.scalar.square(sq_r[:], ps_r[:])
                    sq_i = sqp.tile([P, CH], f32, tag="sqi")
                    nc.scalar.square(sq_i[:], ps_i[:])
                    ssum = sqp.tile([P, CH], f32, tag="ssum")
                    nc.vector.tensor_add(ssum[:], sq_r[:], sq_i[:])
                    nc.scalar.sqrt(out_sb[:, sl], ssum[:])
                nc.sync.dma_start(o_ap[r], out_sb[:])
    nc.compile()
    _prog_cache["nc"] = nc
    return nc


LAST_RESULT = None


def kernel(x_real, x_imag, projection):
    global LAST_RESULT
    from concourse.bass_utils import run_bass_kernel_spmd

    nc = _build_program()
    x_real = np.ascontiguousarray(x_real, dtype=np.float32)
    x_imag = np.ascontiguousarray(x_imag, dtype=np.float32)
    w = np.ascontiguousarray(projection, dtype=np.float32)
    in_maps = []
    for c in range(NCORES):
        sl = slice(c * BC, (c + 1) * BC)
        in_maps.append({
            "xr": np.ascontiguousarray(x_real[sl].transpose(1, 2, 0)),
            "xi": np.ascontiguousarray(x_imag[sl].transpose(1, 2, 0)),
            "w": w,
        })
    res = run_bass_kernel_spmd(nc, in_maps, core_ids=list(range(NCORES)))
    LAST_RESULT = res
    out = np.empty((B, R, P), dtype=np.float32)
    for c in range(NCORES):
        out[c * BC:(c + 1) * BC] = res.results[c]["o"].transpose(2, 0, 1)
    return out


# revision 3
# speedup vs baseline: 1.0649x; 1.0649x over previous
"""Trainium2 Bass kernel for ComplexProjection:
    out[b,r,p] = |sum_s complex(x_real,x_imag)[b,r,s] * projection[r,s,p]|

Strategy: data-parallel over the particle axis B across 8 NeuronCores.
Each core computes, for its B-shard (Bc=4096) and every r:
    re[p,b] = sum_s w[r,s,p] * x_real[b,r,s]   (PE matmul, W stationary)
    im[p,b] = sum_s w[r,s,p] * x_imag[b,r,s]
    out[p,b] = sqrt(re^2 + im^2)               (ACT/DVE epilogue)
The contraction dim S must live on SBUF partitions for both matmul
operands, so the host passes the x shards pre-transposed to [r, s, b]
(cheap numpy work; device time is what counts) and receives the output
as [r, p, b], which the host permutes back.

MODE selects matmul numerics:
  fp32  - native fp32 matmul (4 cyc/row, 2 half-speed passes)
  fp32r - replicated-fp32 matmul (1 cyc/row at N>=512)
"""

import os

import numpy as np

B, R, S, P = 32768, 16, 128, 128
NCORES = 8
BC = B // NCORES  # 4096 particles per core
CH = 512          # matmul moving-dim chunk (one PSUM bank of fp32)
NCH = BC // CH

MODE = os.environ.get("KMODE", "fp32r")
EPI = os.environ.get("KEPI", "dve")  # "act3" (3 ACT) or "dve" (2 ACT + 2 DVE)

_prog_cache = {}


def _build_program():
    key = (MODE, EPI)
    if key in _prog_cache:
        return _prog_cache[key]

    import concourse.tile as tile
    from concourse import bacc, mybir

    f32 = mybir.dt.float32
    xdt = {"fp32": f32, "fp32r": mybir.dt.float32r}[MODE]

    nc = bacc.Bacc("TRN2", target_bir_lowering=False, debug=False,
                   num_devices=NCORES)
    xr = nc.dram_tensor("xr", [R, S, BC], xdt, kind="ExternalInput")
    xi = nc.dram_tensor("xi", [R, S, BC], xdt, kind="ExternalInput")
    w = nc.dram_tensor("w", [R, S, P], xdt, kind="ExternalInput")
    o = nc.dram_tensor("o", [R, P, BC], f32, kind="ExternalOutput")
    xr_ap, xi_ap, w_ap, o_ap = xr.ap(), xi.ap(), w.ap(), o.ap()

    with tile.TileContext(nc) as tc:
        with (
            tc.tile_pool(name="wp", bufs=1) as wp,
            tc.tile_pool(name="xp", bufs=2) as xp,
            tc.tile_pool(name="op", bufs=2) as op,
            tc.tile_pool(name="sq", bufs=3) as sqp,
            tc.tile_pool(name="ps", bufs=2, space="PSUM") as psp,
        ):
            # All R projection matrices stay resident: [s=128, r, p]
            w_sb = wp.tile([S, R, P], xdt)
            for r in range(R):
                nc.sync.dma_start(w_sb[:, r, :], w_ap[r])

            for r in range(R):
                xr_sb = xp.tile([S, BC], xdt, tag="xr")
                nc.sync.dma_start(xr_sb[:], xr_ap[r])
                xi_sb = xp.tile([S, BC], xdt, tag="xi")
                nc.sync.dma_start(xi_sb[:], xi_ap[r])
                out_sb = op.tile([P, BC], f32)
                for c in range(NCH):
                    sl = slice(c * CH, (c + 1) * CH)
                    ps_r = psp.tile([P, CH], f32, tag="psr")
                    nc.tensor.matmul(ps_r[:], w_sb[:, r, :], xr_sb[:, sl],
                                     start=True, stop=True)
                    ps_i = psp.tile([P, CH], f32, tag="psi")
                    nc.tensor.matmul(ps_i[:], w_sb[:, r, :], xi_sb[:, sl],
                                     start=True, stop=True)
                    if EPI == "act3":
                        sq_r = sqp.tile([P, CH], f32, tag="sqr")
                        nc.scalar.square(sq_r[:], ps_r[:])
                        sq_i = sqp.tile([P, CH], f32, tag="sqi")
                        nc.scalar.square(sq_i[:], ps_i[:])
                        ssum = sqp.tile([P, CH], f32, tag="ssum")
                        nc.vector.tensor_add(ssum[:], sq_r[:], sq_i[:])
                        nc.scalar.sqrt(out_sb[:, sl], ssum[:])
                    else:
                        # DVE: copy re to SBUF, square via PSUMxSBUF mult, add.
                        # ACT: square im (PSUM read), final sqrt.
                        # (DVE ops may read at most one PSUM input.)
                        cp_r = sqp.tile([P, CH], f32, tag="cpr")
                        nc.vector.tensor_copy(cp_r[:], ps_r[:])
                        sq_r = sqp.tile([P, CH], f32, tag="sqr")
                        nc.vector.tensor_mul(sq_r[:], ps_r[:], cp_r[:])
                        sq_i = sqp.tile([P, CH], f32, tag="sqi")
                        nc.scalar.square(sq_i[:], ps_i[:])
                        ssum = sqp.tile([P, CH], f32, tag="ssum")
                        nc.vector.tensor_add(ssum[:], sq_r[:], sq_i[:])
                        nc.scalar.sqrt(out_sb[:, sl], ssum[:])
                nc.sync.dma_start(o_ap[r], out_sb[:])
    nc.compile()
    _prog_cache[key] = nc
    return nc


LAST_RESULT = None


def kernel(x_real, x_imag, projection):
    global LAST_RESULT
    from concourse.bass_utils import run_bass_kernel_spmd

    nc = _build_program()
    x_real = np.ascontiguousarray(x_real, dtype=np.float32)
    x_imag = np.ascontiguousarray(x_imag, dtype=np.float32)
    w = np.ascontiguousarray(projection, dtype=np.float32)
    in_maps = []
    for c in range(NCORES):
        sl = slice(c * BC, (c + 1) * BC)
        in_maps.append({
            "xr": np.ascontiguousarray(x_real[sl].transpose(1, 2, 0)),
            "xi": np.ascontiguousarray(x_imag[sl].transpose(1, 2, 0)),
            "w": w,
        })
    res = run_bass_kernel_spmd(nc, in_maps, core_ids=list(range(NCORES)))
    LAST_RESULT = res
    out = np.empty((B, R, P), dtype=np.float32)
    for c in range(NCORES):
        out[c * BC:(c + 1) * BC] = res.results[c]["o"].transpose(2, 0, 1)
    return out
